# revision 18
# baseline (speedup 1.0000x reference)
"""AngProtoLoss (stable) distributed Bass kernel for 8 TRN2 NeuronCores.

Problem (reference):
    dvecs: (4096, 16, 512) f32
    centroids = mean(dvecs, axis=1)                  # (N, D)
    u = dvecs[:, -1, :]                              # (N, D)
    cos = clip(cos_sim(u, centroids), min=1e-6)      # (N, N)
    logits = cos * w + b
    loss = -mean(diag(log_softmax(logits)))
        = mean_i [ logsumexp_k(w*clip(cos_ik)) - w*clip(cos_ii) ]   (b cancels)

Sharding: data-parallel over speakers N; 512 speakers (4 chunks of 128) per
core.

Key structure (v2 — PE-centric):
 - The m-sum (centroid numerator) rides the TensorEngine: 16 accumulating
   float32r matmuls per chunk against a bitcast identity (f32r streams at
   1 cycle/row for free-dim >= 256), instead of a DVE add-tree. This frees
   the DVE (the v1 bottleneck: 100% busy for 45us) and keeps the PE warm.
 - Centroids are normalized (rs_c) before the fp8 transpose+allgather.
   u is NOT normalized: rs_u is folded into the phase-C epilogue as
   per-partition tensor_scalar operands: y = max(ps*rs_u, eps).
 - Per-chunk fp8 AllGather fires as soon as that chunk's cT is ready
   (~2us after its load lands), pipelining the AG chain against the
   remaining loads. Bounce writes + AGs + gathered reads all ride the
   otherwise-empty gpsimd SWDGE ring so they are never queued behind the
   16.8MB of X-load descriptors on the sync HWDGE ring.
 - Phase-C matmul groups are interleaved into the PE FIFO in
   expected-data-arrival order (g0 after chunk2's m-sum, g1 split around
   chunk3's m-sum) so the PE never head-of-line blocks the AG-critical
   m-sums, yet fills its DMA-wait gaps with useful work.
 - Device ships per-(chunk, gather-group) exp-sums and the diagonal cos;
   host does s = sum(parts), rows = log(s) - w*clip(diag), mean.
"""

import os
import sys

for _p in ("/opt/trn_rl_repo",):
    if os.path.isdir(_p) and _p not in sys.path:
        sys.path.append(_p)

import numpy as np

import concourse.bass as bass
import concourse.tile as tile
from concourse import bacc, mybir
from concourse.bass_utils import run_bass_kernel_spmd
from concourse.masks import make_identity

N_CORES = 8
N, M, D = 4096, 16, 512
P = 128                     # partitions
LOCAL = N // N_CORES        # 512 speakers per core
NCHUNK = LOCAL // P         # 4 chunks of 128 speakers
NT = D // P                 # 4 d-tiles
EPS = 1e-6

F32 = mybir.dt.float32
F32R = mybir.dt.float32r
BF16 = mybir.dt.bfloat16
FP8 = mybir.dt.float8e4
AF = mybir.ActivationFunctionType
ALU = mybir.AluOpType


def build_program(w_val: float):
    nc = bacc.Bacc("TRN2", target_bir_lowering=False, debug=False,
                   num_devices=N_CORES)
    # f32r is bit-identical to f32; typing the input chain as f32r lets the
    # m-sum matmuls stream at 1 cycle/row (the verifier requires an all-f32r
    # producer chain for f32r matmul inputs).
    dvecs = nc.dram_tensor("dvecs", [LOCAL, M, D], F32R, kind="ExternalInput").ap()
    out = nc.dram_tensor("out", [5, LOCAL], F32, kind="ExternalOutput").ap()

    with tile.TileContext(nc) as tc:
        _build(nc, tc, dvecs, out, w_val)
    nc.compile()
    return nc


def _build(nc, tc, dvecs, out, w_val):
    from contextlib import ExitStack
    ctx = ExitStack()
    with ctx:
        singles = ctx.enter_context(tc.tile_pool(name="singles", bufs=1))
        xpool = ctx.enter_context(tc.tile_pool(name="xpool", bufs=3))
        cpool = ctx.enter_context(tc.tile_pool(name="cpool", bufs=2))
        scr = ctx.enter_context(tc.tile_pool(name="scr", bufs=3))
        gpool = ctx.enter_context(tc.tile_pool(name="gpool", bufs=1))
        epool = ctx.enter_context(tc.tile_pool(name="epool", bufs=3))
        cpsum = ctx.enter_context(tc.tile_pool(name="cpsum", bufs=2, space="PSUM"))
        tpsum = ctx.enter_context(tc.tile_pool(name="tpsum", bufs=2, space="PSUM"))
        mpsum = ctx.enter_context(tc.tile_pool(name="mpsum", bufs=2, space="PSUM"))
        dram = ctx.enter_context(tc.tile_pool(name="dram", bufs=1, space="DRAM"))

        ident = singles.tile([P, P], F32)
        make_identity(nc, ident)
        ident_bf = singles.tile([P, P], BF16)
        make_identity(nc, ident_bf)
        # f32r identity: the verifier wants f32r matmul inputs produced by a
        # rounding op, so copy the f32 identity through the ACT engine.
        ident_r = singles.tile([P, P], F32R)
        nc.scalar.copy(ident_r, ident)

        # persistent across the whole kernel
        uT = singles.tile([P, NT, LOCAL], BF16)       # u^T (raw): [d, t, i]
        ssq = singles.tile([P, NCHUNK, 2], F32)       # |c|^2, |u|^2
        nrm = singles.tile([P, NCHUNK, 2], F32)       # |c|, |u|
        rs = singles.tile([P, NCHUNK, 2], F32)        # 1/|c|, 1/|u|
        # stats[:, 0:4, q] = partial exp-sums per gather group
        # stats[:, 4, q]  = diag cos
        stats = singles.tile([P, 5, NCHUNK], F32)

        # ---------- phase A: all X loads queue first on the sync ring ----
        xs = []
        for r in range(NCHUNK):
            x = xpool.tile([P, M, D], F32R, name=f"x{r}", tag="x")
            if r == 0:
                # fine-grained so chunk0's m-sum can trail the DMA and its
                # allgather fires earliest
                for j in range(M // 2):
                    nc.sync.dma_start(
                        out=x[:, 2 * j:2 * j + 2, :],
                        in_=dvecs[0:P, 2 * j:2 * j + 2, :])
            else:
                nc.sync.dma_start(out=x, in_=dvecs[r * P:(r + 1) * P, :, :])
            xs.append(x)

        gath = [None] * NCHUNK
        g_sb = [None] * NCHUNK

        # ---------- phase B: per-chunk centroid pipeline + allgather -----
        def emit_chunk(r):
            x = xs[r]
            # m-sum on the PE: csum[i, d] = sum_m x[i, m, d] (f32r @ 1cyc/row)
            csum = cpsum.tile([P, D], F32, name=f"csum{r}", tag="csum")
            for m in range(M):
                nc.tensor.matmul(csum, ident_r, x[:, m, :],
                                 start=(m == 0), stop=(m == M - 1))
            # raw-u transposes (f32r identity moving @ 1.5 cyc/row)
            pu = tpsum.tile([P, NT, P], F32R, name=f"ptu{r}", tag="pt")
            for t in range(NT):
                nc.tensor.transpose(pu[:, t, :], x[:, M - 1, t * P:(t + 1) * P],
                                    ident_r)
            # norms: ssq via ACT Square+accum; sqrt; reciprocal on DVE
            sq_c = scr.tile([P, D], BF16, name=f"sqc{r}", tag="sq")
            nc.scalar.activation(sq_c, csum, AF.Square,
                                 accum_out=ssq[:, r, 0:1])
            sq_u = scr.tile([P, D], BF16, name=f"squ{r}", tag="sq")
            nc.scalar.activation(sq_u, x[:, M - 1, :].bitcast(F32), AF.Square,
                                 accum_out=ssq[:, r, 1:2])
            nc.scalar.activation(nrm[:, r, :], ssq[:, r, :], AF.Sqrt)
            # diag raw: dg = sum_d csum*u (one DVE pass)
            dscr = scr.tile([P, D], BF16, name=f"dscr{r}", tag="sq")
            dg = scr.tile([P, 1], F32, name=f"dg{r}", tag="dg")
            nc.vector.tensor_tensor_reduce(
                out=dscr, in0=csum, in1=x[:, M - 1, :].bitcast(F32), scale=1.0,
                scalar=0.0, op0=ALU.mult, op1=ALU.add, accum_out=dg)
            nc.vector.reciprocal(rs[:, r, :], nrm[:, r, :])
            # normalized centroid (bf16) evicted from PSUM
            chat = cpool.tile([P, D], BF16, name=f"chat{r}", tag="chat")
            nc.vector.tensor_scalar_mul(chat, csum, rs[:, r, 0:1])
            # diag cos = dg * rs_c * rs_u
            nc.vector.tensor_scalar(
                out=stats[:, 4, r:r + 1], in0=dg, scalar1=rs[:, r, 0:1],
                scalar2=rs[:, r, 1:2], op0=ALU.mult, op1=ALU.mult)
            # uT eviction (ACT): after sqrt in the ACT FIFO, ready early
            nc.scalar.copy(uT[:, :, r * P:(r + 1) * P], pu.bitcast(F32))
            # chat transposes (bf16) -> one PSUM bank -> cT fp8
            cT = cpool.tile([P, NT, P], FP8, name=f"cT{r}", tag="cT")
            pc = tpsum.tile([P, NT, P], BF16, name=f"ptc{r}", tag="pt")
            for t in range(NT):
                nc.tensor.transpose(pc[:, t, :], chat[:, t * P:(t + 1) * P],
                                    ident_bf)
            nc.scalar.copy(cT, pc)
            # bounce + allgather on the gpsimd SWDGE ring
            bounce = dram.tile([P, NT * P], FP8, name=f"bounce{r}")
            nc.gpsimd.dma_start(out=bounce,
                                in_=cT.rearrange("p t i -> p (t i)"))
            g = dram.tile([N_CORES * P, NT * P], FP8, name=f"gath{r}",
                          addr_space="Shared")
            nc.gpsimd.collective_compute(
                "AllGather", ALU.bypass,
                replica_groups=[list(range(N_CORES))],
                ins=[bounce.opt()], outs=[g.opt()],
            )
            gath[r] = g

        def emit_gread(gi):
            # gathered fp8 centroids -> SBUF [d, rank, t, i] (gpsimd ring so
            # it queues right behind AG_gi, not behind the X loads)
            g_sb[gi] = gpool.tile([P, N_CORES, NT, P], FP8, name=f"gsb{gi}",
                                  tag=f"gsb{gi}")
            nc.gpsimd.dma_start(
                out=g_sb[gi],
                in_=gath[gi].rearrange("(c p) f -> p c f", p=P).rearrange(
                    "p c (t i) -> p c t i", t=NT))

        def emit_cq(gi, q):
            # 512 queries x 1024 columns for gather group gi, query chunk q
            ps = mpsum.tile([P, 2, 512], F32, name=f"ps{gi}_{q}", tag="ps")
            for h in range(2):
                for t in range(NT):
                    nc.tensor.matmul(
                        ps[:, h, :],
                        uT[:, t, q * P:(q + 1) * P],
                        g_sb[gi][:, 4 * h:4 * h + 4, t, :],
                        start=(t == 0), stop=(t == NT - 1),
                    )
            # y = max(raw*rs_u, eps); s_part = sum_k exp(w*y)
            y = epool.tile([P, 2 * 512], BF16, name=f"y{gi}_{q}", tag="y")
            nc.vector.tensor_scalar(
                out=y, in0=ps.rearrange("p a b -> p (a b)"),
                scalar1=rs[:, q, 1:2], scalar2=EPS,
                op0=ALU.mult, op1=ALU.max)
            e_scr = epool.tile([P, 2 * 512], BF16, name=f"e{gi}_{q}", tag="e")
            nc.scalar.activation(e_scr, y, AF.Exp, scale=w_val,
                                 accum_out=stats[:, gi, q:q + 1])

        # Interleave phase-C blocks into the PE's DMA-wait gaps, but a
        # (gi, q) block may only be emitted after chunk q's B-block (it
        # reads uT[q]) and gread(gi) (it reads g_sb[gi]).
        emit_chunk(0)
        emit_chunk(1)
        emit_gread(0)
        emit_cq(0, 0)
        emit_cq(0, 1)
        emit_chunk(2)
        emit_gread(1)
        emit_cq(0, 2)
        emit_cq(1, 0)
        emit_cq(1, 1)
        emit_chunk(3)
        emit_cq(0, 3)
        emit_cq(1, 2)
        emit_cq(1, 3)
        emit_gread(2)
        for q in range(NCHUNK):
            emit_cq(2, q)
        emit_gread(3)
        for q in range(NCHUNK):
            emit_cq(3, q)

        # ---------- ship everything in one write ----------
        nc.sync.dma_start(out=out.rearrange("k (q p) -> p k q", p=P),
                          in_=stats)


_CACHE = {}


def kernel(dvecs, w, b):
    w_val = float(np.asarray(w))
    key = w_val
    if key not in _CACHE:
        _CACHE[key] = build_program(w_val)
    nc = _CACHE[key]
    dvecs = np.ascontiguousarray(np.asarray(dvecs, dtype=np.float32))
    in_maps = [
        {"dvecs": dvecs[c * LOCAL:(c + 1) * LOCAL]} for c in range(N_CORES)
    ]
    res = run_bass_kernel_spmd(nc, in_maps, core_ids=list(range(N_CORES)))
    total = 0.0
    for c in range(N_CORES):
        o = np.asarray(res.results[c]["out"], dtype=np.float64)
        s = o[0:4].sum(axis=0)
        diag = o[4]
        rows = np.log(s) - w_val * np.maximum(diag, EPS)
        total += float(rows.sum())
    return np.float32(total / N)
